# revision 8
# baseline (speedup 1.0000x reference)
"""Haar DWT 2D (2x2, stride 2) on Trainium2, 8 NeuronCores, pure batch-parallel.

Input  x: (8, 96, 512, 512) f32
Output y: (8, 384, 256, 256) f32  = concat([LL, LH, HL, HH], axis=1), all * 0.25

Core b processes batch b. Per core, the (96, 512, 512) image stack is tiled
into 24 sets of (8 channels x 256 rows). Per set:
  - one 4 MB DMA load: partition p holds rows (2p, 2p+1) -> [128, c=8, 2, 512]
  - DVE: u = even+odd, v = even-odd (full width), then the horizontal
    butterfly (4 half-width ops, stride-2 reads) written back into the load
    tile as per-channel [ll|hl|lh|hh] quarters
  - ACT: scale by 0.25 into U/V tiles
  - four 1 MB DMA stores into the per-subband output blocks
"""

import numpy as np

B, C, H, W = 8, 96, 512, 512
OC, OH, OW = 4 * C, H // 2, W // 2
N_CORES = 8
CH_PER = 8                      # channels per tile-set
N_CGRP = C // CH_PER            # 12
N_HALF = 2                      # 256-row halves per image
FD = CH_PER * W                 # 4096 floats per partition in U/V

_CACHE = {}


def _build_nc():
    import concourse.bacc as bacc
    import concourse.mybir as mybir
    import concourse.tile as tile

    nc = bacc.Bacc("TRN2", target_bir_lowering=False, debug=False)
    x = nc.dram_tensor("x", [C, H, W], mybir.dt.float32, kind="ExternalInput")
    y = nc.dram_tensor("y", [OC, OH, OW], mybir.dt.float32, kind="ExternalOutput")

    # x viewed as [p=128 row-pairs, half, c, (two w)]:  h = half*256 + p*2 + two
    xv = x.ap().rearrange("c (half p two) w -> half p c (two w)",
                          half=N_HALF, p=128, two=2)
    # y viewed as [subband, oh, c, ow]; channel = s*96 + c
    yv = y.ap().rearrange("(s c) oh ow -> s oh c ow", s=4)

    with tile.TileContext(nc) as tc:
        with tc.tile_pool(name="io", bufs=2) as pool:
            for g in range(N_CGRP):
                for half in range(N_HALF):
                    c0 = g * CH_PER
                    r0 = half * 128  # output-row offset for this half
                    T = pool.tile([128, 2 * FD], mybir.dt.float32, tag="T")
                    U = pool.tile([128, FD], mybir.dt.float32, tag="U")
                    V = pool.tile([128, FD], mybir.dt.float32, tag="V")

                    T4 = T[:].rearrange("p (c two w) -> p c two w",
                                        c=CH_PER, two=2)
                    U3 = U[:].rearrange("p (c w) -> p c w", c=CH_PER)
                    V3 = V[:].rearrange("p (c w) -> p c w", c=CH_PER)

                    # load: rows [half*256, half*256+256) of channels c0..c0+8
                    nc.sync.dma_start(
                        out=T4, in_=xv[half, :, c0:c0 + CH_PER, :])

                    ev = T4[:, :, 0, :]   # even rows  [128, 8, 512]
                    od = T4[:, :, 1, :]   # odd rows
                    nc.vector.tensor_add(U3, ev, od)   # u = e + o
                    nc.vector.tensor_sub(V3, ev, od)   # v = e - o

                    ue, uo = U3[:, :, 0::2], U3[:, :, 1::2]
                    ve, vo = V3[:, :, 0::2], V3[:, :, 1::2]
                    # raw subbands back into T: per channel [ll|lh|hl|hh]*256
                    Tq = T[:].rearrange("p (c q w) -> p c q w", c=CH_PER, q=4)
                    nc.vector.tensor_add(Tq[:, :, 0, :], ue, uo)  # ll = ue+uo
                    nc.vector.tensor_add(Tq[:, :, 1, :], ve, vo)  # lh = ve+vo
                    nc.vector.tensor_sub(Tq[:, :, 2, :], ue, uo)  # hl = ue-uo
                    nc.vector.tensor_sub(Tq[:, :, 3, :], ve, vo)  # hh = ve-vo

                    # scale by 0.25 on ScalarE into U (ll|lh) and V (hl|hh)
                    nc.scalar.mul(U3, T4[:, :, 0, :], 0.25)
                    nc.scalar.mul(V3, T4[:, :, 1, :], 0.25)

                    # stores on the ACT HWDGE ring (loads use SP's):
                    # U = [ll|lh], V = [hl|hh] per channel
                    nc.scalar.dma_start(
                        out=yv[0, r0:r0 + 128, c0:c0 + CH_PER, :],
                        in_=U3[:, :, 0:OW])
                    nc.scalar.dma_start(
                        out=yv[1, r0:r0 + 128, c0:c0 + CH_PER, :],
                        in_=U3[:, :, OW:2 * OW])
                    nc.scalar.dma_start(
                        out=yv[2, r0:r0 + 128, c0:c0 + CH_PER, :],
                        in_=V3[:, :, 0:OW])
                    nc.scalar.dma_start(
                        out=yv[3, r0:r0 + 128, c0:c0 + CH_PER, :],
                        in_=V3[:, :, OW:2 * OW])
    nc.compile()
    return nc


def _get_nc():
    if "nc" not in _CACHE:
        _CACHE["nc"] = _build_nc()
    return _CACHE["nc"]


def _run_hw(in_maps):
    """Run the SPMD program on 8 NeuronCores, return per-core output dicts."""
    from concourse import bass2jax
    results = bass2jax.run_bass_via_pjrt(_get_nc(), in_maps, n_cores=N_CORES)
    return results


def kernel(x: np.ndarray) -> np.ndarray:
    x = np.ascontiguousarray(np.asarray(x), dtype=np.float32)
    assert x.shape == (B, C, H, W), x.shape
    in_maps = [{"x": x[b]} for b in range(N_CORES)]
    results = _run_hw(in_maps)
    out = np.stack([results[b]["y"] for b in range(N_CORES)], axis=0)
    return out.astype(np.float32, copy=False)


# revision 10
# speedup vs baseline: 1.2510x; 1.2510x over previous
"""Haar DWT 2D (2x2, stride 2) on Trainium2, 8 NeuronCores, pure batch-parallel.

Input  x: (8, 96, 512, 512) f32
Output y: (8, 384, 256, 256) f32  = concat([LL, LH, HL, HH], axis=1), all * 0.25

Core b processes batch b. Per core, the (96, 512, 512) image stack is tiled
into 24 sets of (8 channels x 256 rows). Per set:
  - one 4 MB DMA load: partition p holds rows (2p, 2p+1) -> [128, c=8, 2, 512]
  - DVE: u = even+odd, v = even-odd (full width), then the horizontal
    butterfly (4 half-width ops, stride-2 reads) written back into the load
    tile as per-channel [ll|hl|lh|hh] quarters
  - ACT: scale by 0.25 into U/V tiles
  - four 1 MB DMA stores into the per-subband output blocks
"""

import numpy as np

B, C, H, W = 8, 96, 512, 512
OC, OH, OW = 4 * C, H // 2, W // 2
N_CORES = 8
CH_PER = 8                      # channels per tile-set
N_CGRP = C // CH_PER            # 12
N_HALF = 2                      # 256-row halves per image
FD = CH_PER * W                 # 4096 floats per partition in U/V

_CACHE = {}


def _build_nc(reps=1):
    import concourse.bacc as bacc
    import concourse.mybir as mybir
    import concourse.tile as tile

    nc = bacc.Bacc("TRN2", target_bir_lowering=False, debug=False)
    x = nc.dram_tensor("x", [C, H, W], mybir.dt.float32, kind="ExternalInput")
    y = nc.dram_tensor("y", [OC, OH, OW], mybir.dt.float32, kind="ExternalOutput")

    # x viewed as [p=128 row-pairs, half, c, (two w)]:  h = half*256 + p*2 + two
    xv = x.ap().rearrange("c (half p two) w -> half p c (two w)",
                          half=N_HALF, p=128, two=2)
    # y viewed as [subband, oh, c, ow]; channel = s*96 + c
    yv = y.ap().rearrange("(s c) oh ow -> s oh c ow", s=4)

    with tile.TileContext(nc) as tc:
        with tc.tile_pool(name="io", bufs=2) as pool:
            for _rep in range(reps):
              for g in range(N_CGRP):
                for half in range(N_HALF):
                    c0 = g * CH_PER
                    r0 = half * 128  # output-row offset for this half
                    T = pool.tile([128, 2 * FD], mybir.dt.float32, tag="T")
                    U = pool.tile([128, FD], mybir.dt.float32, tag="U")
                    V = pool.tile([128, FD], mybir.dt.float32, tag="V")

                    T4 = T[:].rearrange("p (c two w) -> p c two w",
                                        c=CH_PER, two=2)
                    U3 = U[:].rearrange("p (c w) -> p c w", c=CH_PER)
                    V3 = V[:].rearrange("p (c w) -> p c w", c=CH_PER)

                    # load: rows [half*256, half*256+256) of channels c0..c0+8
                    nc.sync.dma_start(
                        out=T4, in_=xv[half, :, c0:c0 + CH_PER, :])

                    ev = T4[:, :, 0, :]   # even rows  [128, 8, 512]
                    od = T4[:, :, 1, :]   # odd rows
                    nc.vector.tensor_add(U3, ev, od)   # u = e + o
                    nc.vector.tensor_sub(V3, ev, od)   # v = e - o

                    ue, uo = U3[:, :, 0::2], U3[:, :, 1::2]
                    ve, vo = V3[:, :, 0::2], V3[:, :, 1::2]
                    # raw subbands back into T: per channel [ll|lh|hl|hh]*256
                    Tq = T[:].rearrange("p (c q w) -> p c q w", c=CH_PER, q=4)
                    nc.vector.tensor_add(Tq[:, :, 0, :], ue, uo)  # ll = ue+uo
                    nc.vector.tensor_add(Tq[:, :, 1, :], ve, vo)  # lh = ve+vo
                    nc.vector.tensor_sub(Tq[:, :, 2, :], ue, uo)  # hl = ue-uo
                    nc.vector.tensor_sub(Tq[:, :, 3, :], ve, vo)  # hh = ve-vo

                    # scale by 0.25 on ScalarE into U (ll|lh) and V (hl|hh)
                    nc.scalar.mul(U3, T4[:, :, 0, :], 0.25)
                    nc.scalar.mul(V3, T4[:, :, 1, :], 0.25)

                    # stores on the ACT HWDGE ring (loads use SP's):
                    # U = [ll|lh], V = [hl|hh] per channel
                    nc.scalar.dma_start(
                        out=yv[0, r0:r0 + 128, c0:c0 + CH_PER, :],
                        in_=U3[:, :, 0:OW])
                    nc.scalar.dma_start(
                        out=yv[1, r0:r0 + 128, c0:c0 + CH_PER, :],
                        in_=U3[:, :, OW:2 * OW])
                    nc.scalar.dma_start(
                        out=yv[2, r0:r0 + 128, c0:c0 + CH_PER, :],
                        in_=V3[:, :, 0:OW])
                    nc.scalar.dma_start(
                        out=yv[3, r0:r0 + 128, c0:c0 + CH_PER, :],
                        in_=V3[:, :, OW:2 * OW])
    nc.compile()
    return nc


def _get_nc():
    if "nc" not in _CACHE:
        _CACHE["nc"] = _build_nc()
    return _CACHE["nc"]


def _run_hw(in_maps):
    """Run the SPMD program on 8 NeuronCores, return per-core output dicts."""
    from concourse import bass2jax
    results = bass2jax.run_bass_via_pjrt(_get_nc(), in_maps, n_cores=N_CORES)
    return results


def kernel(x: np.ndarray) -> np.ndarray:
    x = np.ascontiguousarray(np.asarray(x), dtype=np.float32)
    assert x.shape == (B, C, H, W), x.shape
    in_maps = [{"x": x[b]} for b in range(N_CORES)]
    results = _run_hw(in_maps)
    out = np.stack([results[b]["y"] for b in range(N_CORES)], axis=0)
    return out.astype(np.float32, copy=False)
